# revision 1
# baseline (speedup 1.0000x reference)
import os
import numpy as np
from contextlib import ExitStack

import concourse.bass as bass
import concourse.bacc as bacc
import concourse.mybir as mybir
import concourse.tile as tile
from concourse.bass_utils import run_bass_kernel_spmd

NCORES = 8
B = 8
C = 256
HW = 1024
PL = HW // NCORES  # 128 query positions per core

F32 = mybir.dt.float32
F32R = mybir.dt.float32r


def build_nc(use_f32r=True, finalize=True):
    MD = F32R if use_f32r else F32

    # Bacc (not plain Bass): its compile() pass legalizes sync — multi-wait
    # matmuls move waits onto LdWeights, drains become EventSemaphores.
    # Without it walrus codegen rejects ">1 sync wait" instructions.
    nc = bacc.Bacc(None, target_bir_lowering=False)

    # Inputs (per-core identical except xm):
    #   xg: replicated g-input, layout [kc, c_local, j*8+d]  (col j*8+d, c = kc*128+c_local)
    #   xm: per-core slice, layout [kc, c_local, k*128+p_local]
    #   wg/wm: w_g.T / w_mask.T chunked on contraction axis
    xg_h = nc.declare_dram_parameter("xg", [2, 128, 8192], MD, isOutput=False)
    xm_h = nc.declare_dram_parameter("xm", [2, 128, 1024], MD, isOutput=False)
    wg_h = nc.declare_dram_parameter("wg", [2, 128, 256], MD, isOutput=False)
    wm_h = nc.declare_dram_parameter("wm", [2, 128, 256], MD, isOutput=False)
    out_h = nc.declare_dram_parameter("out", [B, C, PL], F32, isOutput=True)

    with (
        tile.TileContext(nc) as tc,
        ExitStack() as ctx,
    ):
        sb = ctx.enter_context(tc.tile_pool(name="sb", bufs=1))
        dram = ctx.enter_context(tc.tile_pool(name="dram", bufs=1, space="DRAM"))
        # padded to 4KB/32KB: tiny CC payloads fail at runtime
        r_loc = dram.tile([1024], F32, name="r_loc", tag="r_loc")
        r_all = dram.tile([8192], F32, name="r_all", tag="r_all", addr_space="Shared")
        attn_d = dram.tile([1024], F32, name="attn_d", tag="attn_d")
        ps1 = ctx.enter_context(tc.tile_pool(name="ps1", bufs=2, space="PSUM"))
        ps2 = ctx.enter_context(tc.tile_pool(name="ps2", bufs=4, space="PSUM"))
        ps4 = ctx.enter_context(tc.tile_pool(name="ps4", bufs=1, space="PSUM"))

        wgt = [sb.tile([128, 256], MD, name=f"wg{c}", tag=f"wg{c}") for c in range(2)]
        wmt = [sb.tile([128, 256], MD, name=f"wm{c}", tag=f"wm{c}") for c in range(2)]
        xmt = [sb.tile([128, 1024], MD, name=f"xm{c}", tag=f"xm{c}") for c in range(2)]
        xgt = [[sb.tile([128, 2048], MD, name=f"xg{c}_{q}", tag=f"xg{c}_{q}") for q in range(4)] for c in range(2)]
        gt = [sb.tile([128, 8192], MD, name=f"g{c}", tag=f"g{c}") for c in range(2)]
        gm = [sb.tile([128, 1024], MD, name=f"gm{c}", tag=f"gm{c}") for c in range(2)]
        conv = [sb.tile([128, 1024], F32, name=f"conv{c}", tag=f"conv{c}") for c in range(2)]
        gmaxt = [sb.tile([128, 1024], F32, name=f"gmax{t}", tag=f"gmax{t}") for t in range(8)]
        rsb = sb.tile([128, 8], F32, name="rsb", tag="rsb")
        rm8 = sb.tile([8, 128], F32, name="rm8", tag="rm8")
        em = sb.tile([8, 128], F32, name="em", tag="em")
        attn_t = sb.tile([8, 128], F32, name="attn_t", tag="attn_t")
        rsmall = sb.tile([8, 128], F32, name="rsmall", tag="rsmall")
        rt2 = sb.tile([8, 16], F32, name="rt2", tag="rt2")
        corr = sb.tile([8, 8], F32, name="corr", tag="corr")
        prod = sb.tile([8, 8], F32, name="prod", tag="prod")
        lmax = sb.tile([8, 1], F32, name="lmax", tag="lmax")
        negl = sb.tile([8, 1], F32, name="negl", tag="negl")
        lsum = sb.tile([8, 1], F32, name="lsum", tag="lsum")
        gmax = sb.tile([8, 1], F32, name="gmax", tag="gmax")
        negg = sb.tile([8, 1], F32, name="negg", tag="negg")
        gsum = sb.tile([8, 1], F32, name="gsum", tag="gsum")
        rinv = sb.tile([8, 1], F32, name="rinv", tag="rinv")
        myc = sb.tile([8, 1], F32, name="myc", tag="myc")
        sc = sb.tile([8, 1], F32, name="sc", tag="sc")
        attnB = sb.tile([128, 1024], F32, name="attnB", tag="attnB")
        outsb = [gmaxt[0], gmaxt[1]]  # free after the rsb reduce_sums

        # ---- input DMAs ----
        for cc in range(2):
            nc.sync.dma_start(out=wgt[cc][:], in_=wg_h[cc])
            nc.sync.dma_start(out=wmt[cc][:], in_=wm_h[cc])
            nc.sync.dma_start(out=xmt[cc][:], in_=xm_h[cc])
        for q in range(4):
            for cc in range(2):
                nc.sync.dma_start(out=xgt[cc][q][:], in_=xg_h[cc, :, q * 2048:(q + 1) * 2048])

        # ---- phase 1b: gm[c_out, k*128+p] = (w_g @ x_mine)  (per-core g, k-major cols) ----
        for co in range(2):
            for n in range(2):
                pt = ps1.tile([128, 512], F32, name="p1", tag="p1")
                for kc in range(2):
                    nc.tensor.matmul(
                        out=pt[:],
                        lhsT=wgt[kc][:, co * 128:(co + 1) * 128],
                        rhs=xmt[kc][:, n * 512:(n + 1) * 512],
                        start=(kc == 0),
                        stop=(kc == 1),
                    )
                nc.scalar.copy(out=gm[co][:, n * 512:(n + 1) * 512], in_=pt[:])

        # ---- phase 4a: conv = w_mask @ x_mine (attn multiply happens later) ----
        for co in range(2):
            for n in range(2):
                pt = ps4.tile([128, 512], F32, name="p4", tag="p4")
                for kc in range(2):
                    nc.tensor.matmul(
                        out=pt[:],
                        lhsT=wmt[kc][:, co * 128:(co + 1) * 128],
                        rhs=xmt[kc][:, n * 512:(n + 1) * 512],
                        start=(kc == 0),
                        stop=(kc == 1),
                    )
                nc.scalar.copy(out=conv[co][:, n * 512:(n + 1) * 512], in_=pt[:])

        # ---- phases 1a + 2 interleaved per 512-col chunk n ----
        # 1a: gt[c_out, j*8+d] = w_g @ x_all   (global g, (j,d)-interleaved cols)
        # 2:  Gram tile [my 128 i's for batch k=t] x [512 cols of (j,d)] -> grouped max over d
        for n in range(16):
            q, qi = n // 4, n % 4
            for co in range(2):
                pt = ps1.tile([128, 512], F32, name="p1", tag="p1")
                for kc in range(2):
                    nc.tensor.matmul(
                        out=pt[:],
                        lhsT=wgt[kc][:, co * 128:(co + 1) * 128],
                        rhs=xgt[kc][q][:, qi * 512:(qi + 1) * 512],
                        start=(kc == 0),
                        stop=(kc == 1),
                    )
                nc.scalar.copy(out=gt[co][:, n * 512:(n + 1) * 512], in_=pt[:])
            for t in range(8):
                pt = ps2.tile([128, 512], F32, name="p2", tag="p2")
                for kc in range(2):
                    nc.tensor.matmul(
                        out=pt[:],
                        lhsT=gm[kc][:, t * 128:(t + 1) * 128],
                        rhs=gt[kc][:, n * 512:(n + 1) * 512],
                        start=(kc == 0),
                        stop=(kc == 1),
                    )
                dst = gmaxt[t][:, n * 64:(n + 1) * 64]
                nc.vector.reduce_max(
                    out=dst,
                    in_=pt[:].rearrange("p (j e) -> p j e", e=8),
                    axis=mybir.AxisListType.X,
                )
                if n == 15:
                    # row sum for batch t ready as soon as its last chunk lands
                    if t % 2 == 0:
                        nc.vector.reduce_sum(
                            out=rsb[:, t:t + 1], in_=gmaxt[t][:],
                            axis=mybir.AxisListType.X,
                        )
                    else:
                        nc.scalar.activation(
                            out=attnB[:], in_=gmaxt[t][:],
                            func=mybir.ActivationFunctionType.Copy,
                            accum_out=rsb[:, t:t + 1],
                        )
                    # scatter this column now so the transpose round trip
                    # overlaps the remaining row sums
                    rl = r_loc[:]
                    nc.gpsimd.dma_start(
                        out=bass.AP(tensor=rl.tensor, offset=rl.offset + t * 128,
                                    ap=[[1, 128], [1, 1]]),
                        in_=rsb[:, t:t + 1],
                    )

        # ---- transpose rsb [128,8] -> rm8 [8,128] via DRAM round trip ----
        nc.gpsimd.dma_start(out=rm8[:], in_=r_loc[:].rearrange("(k p) -> k p", k=8))

        # ---- local softmax stats (two-phase softmax) ----
        nc.vector.reduce_max(out=lmax[:], in_=rm8[:], axis=mybir.AxisListType.X)
        nc.vector.tensor_scalar_mul(out=negl[:], in0=lmax[:], scalar1=-1.0 / 128.0)
        nc.scalar.activation(
            out=em[:], in_=rm8[:], func=mybir.ActivationFunctionType.Exp,
            bias=negl[:], scale=1.0 / 128.0, accum_out=lsum[:],
        )
        nc.vector.tensor_copy(rsmall[:], em[:])
        nc.vector.tensor_copy(rsmall[:, 0:1], lmax[:])
        nc.vector.tensor_copy(rsmall[:, 1:2], lsum[:])

        # ---- AllGather 16 floats (lmax|lsum per batch) across cores ----
        core_ids = list(range(NCORES))
        nc.gpsimd.dma_start(
            out=r_loc[:].rearrange("(k c) -> k c", c=128), in_=rsmall[:],
        )
        nc.gpsimd.collective_compute(
            "AllGather",
            mybir.AluOpType.bypass,
            replica_groups=[core_ids],
            ins=[r_loc[:].opt()],
            outs=[r_all[:].opt()],
        )
        # r_all layout: [r*1024 + k*128 + c], c in 0..1 -> rt2[k, r*2+c]
        ra = r_all[:]
        nc.gpsimd.dma_start(
            out=rt2[:].rearrange("k (r c) -> k r c", c=2),
            in_=bass.AP(tensor=ra.tensor, offset=ra.offset,
                        ap=[[128, 8], [1024, 8], [1, 2]]),
        )

        # ---- combine: gmax/gsum from 8 cores' (lmax, lsum) ----
        a = rt2[:]
        lmaxl = bass.AP(tensor=a.tensor, offset=a.offset, ap=[a.ap[0], [2, 8]])
        lsuml = bass.AP(tensor=a.tensor, offset=a.offset + 1, ap=[a.ap[0], [2, 8]])
        nc.vector.reduce_max(out=gmax[:], in_=lmaxl, axis=mybir.AxisListType.X)
        nc.vector.tensor_scalar_mul(out=negg[:], in0=gmax[:], scalar1=-1.0 / 128.0)
        nc.scalar.activation(
            out=corr[:], in_=lmaxl, func=mybir.ActivationFunctionType.Exp,
            bias=negg[:], scale=1.0 / 128.0,
        )
        nc.vector.tensor_mul(out=prod[:], in0=corr[:], in1=lsuml)
        nc.vector.reduce_sum(out=gsum[:], in_=prod[:], axis=mybir.AxisListType.X)
        nc.vector.reciprocal(out=rinv[:], in_=gsum[:])
        nc.scalar.activation(
            out=myc[:], in_=lmax[:], func=mybir.ActivationFunctionType.Exp,
            bias=negg[:], scale=1.0 / 128.0,
        )
        nc.vector.tensor_mul(out=sc[:], in0=myc[:], in1=rinv[:])
        nc.vector.tensor_scalar_mul(out=attn_t[:], in0=em[:], scalar1=sc[:])

        # broadcast attn over 128 partitions: attnB[p, k*128+m] = attn[k, m]
        nc.gpsimd.dma_start(out=attn_d[:].rearrange("(k p) -> k p", k=8), in_=attn_t[:])
        ad = attn_d[:]
        bcast = bass.AP(tensor=ad.tensor, offset=ad.offset, ap=[[0, 128], ad.ap[0]])
        nc.gpsimd.dma_start(out=attnB[:], in_=bcast)

        # ---- final: out = conv * attn, DMA out ----
        for co in range(2):
            nc.vector.tensor_mul(out=outsb[co][:], in0=conv[co][:], in1=attnB[:])
            nc.sync.dma_start(
                out=out_h[:, co * 128:(co + 1) * 128, :].rearrange("k co p -> co k p"),
                in_=outsb[co][:].rearrange("co (k p) -> co k p", k=8),
            )

    if finalize:
        nc.finalize()
    return nc


def _prep_inputs(x, w_g, w_mask):
    xr = x.reshape(B, C, HW)
    # xg cols: j*8+d  (j = pixel, d = batch), rows c
    xg = np.ascontiguousarray(xr.transpose(1, 2, 0)).reshape(2, 128, 8192)
    wg = np.ascontiguousarray(w_g.T).reshape(2, 128, 256)
    wm = np.ascontiguousarray(w_mask.T).reshape(2, 128, 256)
    in_maps = []
    for r in range(NCORES):
        xs = xr[:, :, r * PL:(r + 1) * PL]
        # xm cols: k*128 + p_local, rows c
        xm = np.ascontiguousarray(xs.transpose(1, 0, 2)).reshape(2, 128, 1024)
        in_maps.append({"xg": xg, "xm": xm, "wg": wg, "wm": wm})
    return in_maps


def kernel(**inputs):
    x = np.ascontiguousarray(inputs["x"], dtype=np.float32)
    w_g = np.ascontiguousarray(inputs["w_g"], dtype=np.float32)
    w_mask = np.ascontiguousarray(inputs["w_mask"], dtype=np.float32)

    in_maps = _prep_inputs(x, w_g, w_mask)
    nc = build_nc(use_f32r=os.environ.get("KERNEL_NO_F32R", "0") != "1")
    trace = os.environ.get("KERNEL_TRACE", "0") == "1"
    res = run_bass_kernel_spmd(nc, in_maps, list(range(NCORES)), trace=trace)
    globals()["_last_exec_time_ns"] = getattr(res, "exec_time_ns", None)
    outs = [res.results[i]["out"] for i in range(NCORES)]
    return np.concatenate(outs, axis=2).reshape(B, C, 32, 32).astype(np.float32)

